# Initial kernel scaffold
#
"""Trainium2 Bass kernel for nn_BoneLinear: out = x @ W^T + pooled(x) @ disha.

Identity used: pooled(x) @ disha == x @ A where A[j, o] = disha[j % 64, o]
(vertical tiling of disha).  So the whole module is one dense matmul:
    out = x @ (W^T + tile(disha, 16))

Per-core pipeline (all 8 cores run this over their own batch shard):
  1. Setup: load W naturally, PE-transpose it (4 transposes packed per PSUM
     bank as one accumulation group), add the partition-tiled disha, and round
     to fp16 -> W_eff^T resident in SBUF [128, 8, 1024].
  2. Steady state, software-pipelined over 32 token tiles: HWDGE-load x in
     1 MB pairs -> DVE cast f32->fp16 -> PE-transpose each [128,128] chunk
     (packed 4/bank) -> DVE copy to SBUF -> 16 accumulating fp16 matmuls
     (N=512, fp16 gets fast-weight-load + 2 elem/cycle moving stream) ->
     scheduler-balanced ACT/DVE copies PSUM->SBUF -> HWDGE store.
     Staging pools are 4-6 deep so DMA bursts and copies stay off the PE
     critical path.
  fp16 operands measured relmax ~3.3e-4 vs the fp32 reference (fp32 matmul
  proper is 4 cyc/row = ~4x slower; float32r is 1 cyc/row but pays a 4-byte
  weight load per matmul).

Sharding: pure data-parallel over batch (B=8 -> one batch element per core).
Each core reads its x shard [4096, 1024], full weight and disha, and writes
its output shard [4096, 1024].  No collectives.
"""

import sys
import os

for _p in ("/opt/trn_rl_repo", "/root/.axon_site/_ro/trn_rl_repo"):
    if os.path.isdir(_p) and _p not in sys.path:
        sys.path.insert(0, _p)

import numpy as np

import concourse.bass as bass
import concourse.mybir as mybir
import concourse.tile as tile
from concourse import bacc
from concourse.bass_utils import run_bass_kernel_spmd
from concourse.masks import make_identity

# Problem shapes (hardcoded per contract)
B, S, D_IN, D_OUT, R = 8, 4096, 1024, 1024, 64
N_CORES = 8
P = 128
KO = D_IN // P          # 8 contraction chunks of 128
OC = D_OUT // P         # 8 output chunks of 128 (for W transpose)
MT = S // P             # 32 token tiles per core
NF = 512                # matmul moving free dim (one PSUM bank of fp32)
NT = D_OUT // NF        # 2 n-tiles

F32 = mybir.dt.float32
F32R = mybir.dt.float32r
F16 = mybir.dt.float16
# matmul operand dtype: fp16 gets fast-weight-load (2 elem/32-bit read) and
# 1 cyc/row transposes; measured relmax vs fp64 reference ~2.9e-4.
MM_DT = F16


def build_bass(reps: int = 1, loop: int = 1, cast_any: bool = True,
               pair_loads: bool = True, tp_bufs: int = 4, acc_bufs: int = 2,
               n_outer: bool = False, batch_out: bool = False,
               any_out: bool = True, deep: bool = True):
    """reps>1 (python-unrolled) or loop>1 (hardware For_i) repeat the
    steady-state compute inside the NEFF; used only for wall-clock
    differencing in benchmarks (the graded kernel uses reps=1, loop=1)."""
    cast_copy = nc_any_copy = None  # set below once nc exists
    nc = bacc.Bacc("TRN2", target_bir_lowering=False, debug=False, num_devices=1)
    x_ap = nc.dram_tensor("x", [S, D_IN], F32, kind="ExternalInput").ap()
    w_ap = nc.dram_tensor("w", [D_OUT, D_IN], F32, kind="ExternalInput").ap()
    d_ap = nc.dram_tensor("disha", [R, D_OUT], F32, kind="ExternalInput").ap()
    out_ap = nc.dram_tensor("out", [S, D_OUT], F32, kind="ExternalOutput").ap()

    with tile.TileContext(nc) as tc:
        with (
            tc.tile_pool(name="const", bufs=1) as const,
            tc.tile_pool(name="wp", bufs=1) as wpool,
            tc.tile_pool(name="xp", bufs=6 if deep else 4) as xpool,
            tc.tile_pool(name="xh", bufs=4 if deep else 3) as xhpool,
            tc.tile_pool(name="xtp", bufs=4 if deep else 3) as xtpool,
            tc.tile_pool(name="op", bufs=4 if deep else 3) as opool,
            tc.tile_pool(name="pstp", bufs=tp_bufs, space="PSUM") as psum_tp,
            tc.tile_pool(name="psacc", bufs=acc_bufs, space="PSUM") as psum_acc,
        ):
            ident = const.tile([P, P], MM_DT)
            make_identity(nc, ident)

            # disha tiled twice on partitions: disha2[p, :] = disha[p % 64, :]
            disha2f = const.tile([P, D_OUT], F32)
            nc.sync.dma_start(disha2f[0:R, :], d_ap[:, :])
            nc.sync.dma_start(disha2f[R : 2 * R, :], d_ap[:, :])
            cast_copy = nc.any.tensor_copy if cast_any else nc.vector.tensor_copy
            disha2 = const.tile([P, D_OUT], MM_DT)
            cast_copy(disha2[:], disha2f[:])

            # Build W_eff^T[p + 128*kc, oc*128 + q] = W[q(within oc), p(of kc)] + disha2[p]
            # 4 PE transposes packed per PSUM bank (one accumulation group),
            # then a single wide DVE add per bank.
            GRP = NF // P  # 4 transposes per bank
            w_eff = wpool.tile([P, KO, D_OUT], MM_DT)
            with tc.tile_pool(name="wnat", bufs=1) as wnat_pool:
                w_nat = wnat_pool.tile([P, OC, D_IN], F32)
                w_nath = wnat_pool.tile([P, OC, D_IN], MM_DT)
                w_src = w_ap.rearrange("(oc p) d -> p oc d", p=P)
                for kc in range(KO):
                    nc.sync.dma_start(
                        w_nat[:, :, kc * P : (kc + 1) * P],
                        w_src[:, :, kc * P : (kc + 1) * P],
                    )
                    cast_copy(
                        w_nath[:, :, kc * P : (kc + 1) * P],
                        w_nat[:, :, kc * P : (kc + 1) * P],
                    )
                for kc in range(KO):
                    for og in range(OC // GRP):
                        pst = psum_tp.tile([P, NF], MM_DT, tag="tp")
                        for j in range(GRP):
                            oc = og * GRP + j
                            nc.tensor.matmul(
                                pst[:, j * P : (j + 1) * P],
                                w_nath[:, oc, kc * P : (kc + 1) * P],
                                ident[:],
                                is_transpose=True,
                                start=(j == 0),
                                stop=(j == GRP - 1),
                            )
                        nc.vector.tensor_add(
                            w_eff[:, kc, og * NF : (og + 1) * NF],
                            pst[:],
                            disha2[:, og * NF : (og + 1) * NF],
                        )

            # Main loop over token tiles
            import contextlib

            loop_cm = (
                tc.For_i(0, loop, 1) if loop > 1 else contextlib.nullcontext()
            )
            with loop_cm:
                for rep in range(reps):

                    PW = 2 if pair_loads else 1

                    def emit_load_pair(mp, rep=rep):
                        """DMA PW token tiles at once and cast to fp16."""
                        x_t = xpool.tile(
                            [P, PW, D_IN], F32, tag="x_t", name=f"x_{rep}_{mp}"
                        )
                        nc.sync.dma_start(
                            x_t[:],
                            x_ap[mp * PW * P : (mp + 1) * PW * P, :].rearrange(
                                "(two p) d -> p two d", two=PW
                            ),
                        )
                        x_h = xhpool.tile(
                            [P, PW, D_IN], MM_DT, tag="x_h", name=f"xh_{rep}_{mp}"
                        )
                        cast_copy(x_h[:], x_t[:])
                        return x_h

                    def emit_transpose(x_h, t, m, rep=rep):
                        """PE-transpose token tile m (= half t of a pair)."""
                        xT = xtpool.tile(
                            [P, KO, P], MM_DT, tag="xT", name=f"xT_{rep}_{m}"
                        )
                        for g in range(KO // GRP):
                            pst = psum_tp.tile(
                                [P, NF], MM_DT, tag="tp", name=f"tp_{rep}_{m}_{g}"
                            )
                            for j in range(GRP):
                                kc = g * GRP + j
                                nc.tensor.matmul(
                                    pst[:, j * P : (j + 1) * P],
                                    x_h[:, t, kc * P : (kc + 1) * P],
                                    ident[:],
                                    is_transpose=True,
                                    start=(j == 0),
                                    stop=(j == GRP - 1),
                                )
                            nc.vector.tensor_copy(
                                xT[:, g * GRP : (g + 1) * GRP], pst[:]
                            )
                        return xT

                    xh_cur = emit_load_pair(0)
                    xT_cur = emit_transpose(xh_cur, 0, 0)
                    xh_pairs = {0: xh_cur}
                    for m in range(MT):
                        # prefetch pair + transpose next tile before this tile's MMs
                        if (m + 1) % PW == 0 and (m + 1) // PW < MT // PW:
                            xh_pairs[(m + 1) // PW] = emit_load_pair((m + 1) // PW)
                        xT_next = (
                            emit_transpose(
                                xh_pairs[(m + 1) // PW], (m + 1) % PW, m + 1
                            )
                            if m + 1 < MT
                            else None
                        )

                        if batch_out:
                            if m % 2 == 0:
                                o_sb2 = opool.tile(
                                    [P, 2, D_OUT], F32, tag="o2", name=f"o2_{rep}_{m}"
                                )
                            o_sb = o_sb2[:, m % 2, :]
                        else:
                            o_sb = opool.tile([P, D_OUT], F32)
                        pss = [
                            psum_acc.tile(
                                [P, NF], F32, tag=f"acc{n}", name=f"acc_{rep}_{m}_{n}"
                            )
                            for n in range(NT)
                        ]
                        mm_order = (
                            [(kc, n) for n in range(NT) for kc in range(KO)]
                            if n_outer
                            else [(kc, n) for kc in range(KO) for n in range(NT)]
                        )
                        for kc, n in mm_order:
                            nc.tensor.matmul(
                                pss[n][:],
                                xT_cur[:, kc],
                                w_eff[:, kc, n * NF : (n + 1) * NF],
                                start=(kc == 0),
                                stop=(kc == KO - 1),
                            )
                        out_copy = (
                            nc.any.tensor_copy if any_out else nc.scalar.copy
                        )
                        for n in range(NT):
                            out_copy(
                                o_sb[:, n * NF : (n + 1) * NF], pss[n][:]
                            )
                        if batch_out:
                            if m % 2 == 1:
                                nc.sync.dma_start(
                                    out_ap[(m - 1) * P : (m + 1) * P, :].rearrange(
                                        "(two p) d -> p two d", two=2
                                    ),
                                    o_sb2[:],
                                )
                        else:
                            nc.sync.dma_start(
                                out_ap[m * P : (m + 1) * P, :], o_sb[:]
                            )
                        xT_cur = xT_next

    nc.compile()
    return nc


def kernel(x: np.ndarray, weight: np.ndarray, disha: np.ndarray) -> np.ndarray:
    assert x.shape == (B, S, D_IN) and weight.shape == (D_OUT, D_IN)
    assert disha.shape == (R, D_OUT)
    x = np.ascontiguousarray(x, dtype=np.float32)
    weight = np.ascontiguousarray(weight, dtype=np.float32)
    disha = np.ascontiguousarray(disha, dtype=np.float32)
    in_maps = [
        {"x": x[c], "w": weight, "disha": disha} for c in range(N_CORES)
    ]
    # The axon-proxied exec occasionally dies with NRT_EXEC_UNIT_UNRECOVERABLE
    # on an otherwise-good NEFF; retry a couple of times with a fresh build.
    last_exc = None
    for attempt in range(3):
        try:
            nc = build_bass()
            res = run_bass_kernel_spmd(
                nc, in_maps, core_ids=list(range(N_CORES))
            )
            break
        except Exception as e:  # noqa: BLE001
            last_exc = e
            import time as _time

            _time.sleep(5.0 * (attempt + 1))
    else:
        raise last_exc
    out = np.stack([res.results[c]["out"] for c in range(N_CORES)], axis=0)
    return out


if __name__ == "__main__":
    rng = np.random.default_rng(0)
    x = rng.standard_normal((B, S, D_IN), dtype=np.float32)
    w = (rng.standard_normal((D_OUT, D_IN), dtype=np.float32) / 32.0).astype(
        np.float32
    )
    d = (rng.standard_normal((R, D_OUT), dtype=np.float32) * 0.01).astype(np.float32)
    out = kernel(x=x, weight=w, disha=d)
    print(out.shape, out.dtype)



# revision 1
# speedup vs baseline: 1.1881x; 1.1881x over previous
"""Trainium2 Bass kernel for nn_BoneLinear: out = x @ W^T + pooled(x) @ disha.

Identity used: pooled(x) @ disha == x @ A where A[j, o] = disha[j % 64, o]
(vertical tiling of disha).  So the whole module is one dense matmul:
    out = x @ (W^T + tile(disha, 16))

Per-core pipeline (all 8 cores run this over their own batch shard):
  1. Setup: load W naturally, PE-transpose it (4 transposes packed per PSUM
     bank as one accumulation group), add the partition-tiled disha, and round
     to fp16 -> W_eff^T resident in SBUF [128, 8, 1024].
  2. Steady state, software-pipelined over 32 token tiles: HWDGE-load x in
     1 MB pairs -> DVE cast f32->fp16 -> PE-transpose each [128,128] chunk
     (packed 4/bank) -> DVE copy to SBUF -> 16 accumulating fp16 matmuls
     (N=512, fp16 gets fast-weight-load + 2 elem/cycle moving stream) ->
     scheduler-balanced ACT/DVE copies PSUM->SBUF -> HWDGE store.
     Staging pools are 4-6 deep so DMA bursts and copies stay off the PE
     critical path.
  fp16 operands measured relmax ~3.3e-4 vs the fp32 reference (fp32 matmul
  proper is 4 cyc/row = ~4x slower; float32r is 1 cyc/row but pays a 4-byte
  weight load per matmul).

Sharding: pure data-parallel over batch (B=8 -> one batch element per core).
Each core reads its x shard [4096, 1024], full weight and disha, and writes
its output shard [4096, 1024].  No collectives.
"""

import sys
import os

for _p in ("/opt/trn_rl_repo", "/root/.axon_site/_ro/trn_rl_repo"):
    if os.path.isdir(_p) and _p not in sys.path:
        sys.path.insert(0, _p)

import numpy as np

import concourse.bass as bass
import concourse.mybir as mybir
import concourse.tile as tile
from concourse import bacc
from concourse.bass_utils import run_bass_kernel_spmd
from concourse.masks import make_identity

# Problem shapes (hardcoded per contract)
B, S, D_IN, D_OUT, R = 8, 4096, 1024, 1024, 64
N_CORES = 8
P = 128
KO = D_IN // P          # 8 contraction chunks of 128
OC = D_OUT // P         # 8 output chunks of 128 (for W transpose)
MT = S // P             # 32 token tiles per core
NF = 512                # matmul moving free dim (one PSUM bank of fp32)
NT = D_OUT // NF        # 2 n-tiles

F32 = mybir.dt.float32
F32R = mybir.dt.float32r
F16 = mybir.dt.float16
# matmul operand dtype: fp16 gets fast-weight-load (2 elem/32-bit read) and
# 1 cyc/row transposes; measured relmax vs fp64 reference ~2.9e-4.
MM_DT = F16


def build_bass(reps: int = 1, loop: int = 1, cast_any: bool = True,
               pair_loads: bool = True, tp_bufs: int = 4, acc_bufs: int = 2,
               n_outer: bool = False, batch_out: bool = False,
               any_out: bool = True, deep: bool = True):
    """reps>1 (python-unrolled) or loop>1 (hardware For_i) repeat the
    steady-state compute inside the NEFF; used only for wall-clock
    differencing in benchmarks (the graded kernel uses reps=1, loop=1)."""
    cast_copy = nc_any_copy = None  # set below once nc exists
    nc = bacc.Bacc("TRN2", target_bir_lowering=False, debug=False, num_devices=1)
    x_ap = nc.dram_tensor("x", [S, D_IN], F32, kind="ExternalInput").ap()
    w_ap = nc.dram_tensor("w", [D_OUT, D_IN], F32, kind="ExternalInput").ap()
    d_ap = nc.dram_tensor("disha", [R, D_OUT], F32, kind="ExternalInput").ap()
    out_ap = nc.dram_tensor("out", [S, D_OUT], F32, kind="ExternalOutput").ap()

    with tile.TileContext(nc) as tc:
        with (
            tc.tile_pool(name="const", bufs=1) as const,
            tc.tile_pool(name="wp", bufs=1) as wpool,
            tc.tile_pool(name="xp", bufs=6 if deep else 4) as xpool,
            tc.tile_pool(name="xh", bufs=4 if deep else 3) as xhpool,
            tc.tile_pool(name="xtp", bufs=4 if deep else 3) as xtpool,
            tc.tile_pool(name="op", bufs=4 if deep else 3) as opool,
            tc.tile_pool(name="pstp", bufs=tp_bufs, space="PSUM") as psum_tp,
            tc.tile_pool(name="psacc", bufs=acc_bufs, space="PSUM") as psum_acc,
        ):
            ident = const.tile([P, P], MM_DT)
            make_identity(nc, ident)

            # disha tiled twice on partitions: disha2[p, :] = disha[p % 64, :]
            disha2f = const.tile([P, D_OUT], F32)
            nc.sync.dma_start(disha2f[0:R, :], d_ap[:, :])
            nc.sync.dma_start(disha2f[R : 2 * R, :], d_ap[:, :])
            cast_copy = nc.any.tensor_copy if cast_any else nc.vector.tensor_copy
            disha2 = const.tile([P, D_OUT], MM_DT)
            cast_copy(disha2[:], disha2f[:])

            # Build W_eff^T[p + 128*kc, oc*128 + q] = W[q(within oc), p(of kc)] + disha2[p]
            # 4 PE transposes packed per PSUM bank (one accumulation group),
            # then a single wide DVE add per bank.
            GRP = NF // P  # 4 transposes per bank
            w_eff = wpool.tile([P, KO, D_OUT], MM_DT)
            with tc.tile_pool(name="wnat", bufs=1) as wnat_pool:
                w_nat = wnat_pool.tile([P, OC, D_IN], F32)
                w_nath = wnat_pool.tile([P, OC, D_IN], MM_DT)
                w_src = w_ap.rearrange("(oc p) d -> p oc d", p=P)
                for kc in range(KO):
                    nc.sync.dma_start(
                        w_nat[:, :, kc * P : (kc + 1) * P],
                        w_src[:, :, kc * P : (kc + 1) * P],
                    )
                    cast_copy(
                        w_nath[:, :, kc * P : (kc + 1) * P],
                        w_nat[:, :, kc * P : (kc + 1) * P],
                    )
                for kc in range(KO):
                    for og in range(OC // GRP):
                        pst = psum_tp.tile([P, NF], MM_DT, tag="tp")
                        for j in range(GRP):
                            oc = og * GRP + j
                            nc.tensor.matmul(
                                pst[:, j * P : (j + 1) * P],
                                w_nath[:, oc, kc * P : (kc + 1) * P],
                                ident[:],
                                is_transpose=True,
                                start=(j == 0),
                                stop=(j == GRP - 1),
                            )
                        nc.vector.tensor_add(
                            w_eff[:, kc, og * NF : (og + 1) * NF],
                            pst[:],
                            disha2[:, og * NF : (og + 1) * NF],
                        )

            # Main loop over token tiles
            import contextlib

            loop_cm = (
                tc.For_i(0, loop, 1) if loop > 1 else contextlib.nullcontext()
            )
            with loop_cm:
                for rep in range(reps):

                    PW = 2 if pair_loads else 1

                    def emit_load_pair(mp, rep=rep):
                        """DMA PW token tiles at once and cast to fp16."""
                        x_t = xpool.tile(
                            [P, PW, D_IN], F32, tag="x_t", name=f"x_{rep}_{mp}"
                        )
                        nc.sync.dma_start(
                            x_t[:],
                            x_ap[mp * PW * P : (mp + 1) * PW * P, :].rearrange(
                                "(two p) d -> p two d", two=PW
                            ),
                        )
                        x_h = xhpool.tile(
                            [P, PW, D_IN], MM_DT, tag="x_h", name=f"xh_{rep}_{mp}"
                        )
                        cast_copy(x_h[:], x_t[:])
                        return x_h

                    def emit_transpose(x_h, t, m, rep=rep):
                        """PE-transpose token tile m (= half t of a pair)."""
                        xT = xtpool.tile(
                            [P, KO, P], MM_DT, tag="xT", name=f"xT_{rep}_{m}"
                        )
                        for g in range(KO // GRP):
                            pst = psum_tp.tile(
                                [P, NF], MM_DT, tag="tp", name=f"tp_{rep}_{m}_{g}"
                            )
                            for j in range(GRP):
                                kc = g * GRP + j
                                nc.tensor.matmul(
                                    pst[:, j * P : (j + 1) * P],
                                    x_h[:, t, kc * P : (kc + 1) * P],
                                    ident[:],
                                    is_transpose=True,
                                    start=(j == 0),
                                    stop=(j == GRP - 1),
                                )
                            nc.vector.tensor_copy(
                                xT[:, g * GRP : (g + 1) * GRP], pst[:]
                            )
                        return xT

                    xh_cur = emit_load_pair(0)
                    xT_cur = emit_transpose(xh_cur, 0, 0)
                    xh_pairs = {0: xh_cur}
                    for m in range(MT):
                        # prefetch pair + transpose next tile before this tile's MMs
                        if (m + 1) % PW == 0 and (m + 1) // PW < MT // PW:
                            xh_pairs[(m + 1) // PW] = emit_load_pair((m + 1) // PW)
                        xT_next = (
                            emit_transpose(
                                xh_pairs[(m + 1) // PW], (m + 1) % PW, m + 1
                            )
                            if m + 1 < MT
                            else None
                        )

                        if batch_out:
                            if m % 2 == 0:
                                o_sb2 = opool.tile(
                                    [P, 2, D_OUT], F32, tag="o2", name=f"o2_{rep}_{m}"
                                )
                            o_sb = o_sb2[:, m % 2, :]
                        else:
                            o_sb = opool.tile([P, D_OUT], F32)
                        pss = [
                            psum_acc.tile(
                                [P, NF], F32, tag=f"acc{n}", name=f"acc_{rep}_{m}_{n}"
                            )
                            for n in range(NT)
                        ]
                        mm_order = (
                            [(kc, n) for n in range(NT) for kc in range(KO)]
                            if n_outer
                            else [(kc, n) for kc in range(KO) for n in range(NT)]
                        )
                        for kc, n in mm_order:
                            nc.tensor.matmul(
                                pss[n][:],
                                xT_cur[:, kc],
                                w_eff[:, kc, n * NF : (n + 1) * NF],
                                start=(kc == 0),
                                stop=(kc == KO - 1),
                            )
                        out_copy = (
                            nc.any.tensor_copy if any_out else nc.scalar.copy
                        )
                        for n in range(NT):
                            out_copy(
                                o_sb[:, n * NF : (n + 1) * NF], pss[n][:]
                            )
                        if batch_out:
                            if m % 2 == 1:
                                nc.sync.dma_start(
                                    out_ap[(m - 1) * P : (m + 1) * P, :].rearrange(
                                        "(two p) d -> p two d", two=2
                                    ),
                                    o_sb2[:],
                                )
                        else:
                            nc.sync.dma_start(
                                out_ap[m * P : (m + 1) * P, :], o_sb[:]
                            )
                        xT_cur = xT_next

    nc.compile()
    return nc


def kernel(x: np.ndarray, weight: np.ndarray, disha: np.ndarray) -> np.ndarray:
    assert x.shape == (B, S, D_IN) and weight.shape == (D_OUT, D_IN)
    assert disha.shape == (R, D_OUT)
    x = np.ascontiguousarray(x, dtype=np.float32)
    weight = np.ascontiguousarray(weight, dtype=np.float32)
    disha = np.ascontiguousarray(disha, dtype=np.float32)
    in_maps = [
        {"x": x[c], "w": weight, "disha": disha} for c in range(N_CORES)
    ]
    # The axon-proxied exec occasionally dies with NRT_EXEC_UNIT_UNRECOVERABLE
    # on an otherwise-good NEFF; retry a couple of times with a fresh build.
    last_exc = None
    for attempt in range(3):
        try:
            nc = build_bass()
            res = run_bass_kernel_spmd(
                nc, in_maps, core_ids=list(range(N_CORES))
            )
            break
        except Exception as e:  # noqa: BLE001
            last_exc = e
            import time as _time

            _time.sleep(5.0 * (attempt + 1))
    else:
        raise last_exc
    out = np.stack([res.results[c]["out"] for c in range(N_CORES)], axis=0)
    return out


if __name__ == "__main__":
    rng = np.random.default_rng(0)
    x = rng.standard_normal((B, S, D_IN), dtype=np.float32)
    w = (rng.standard_normal((D_OUT, D_IN), dtype=np.float32) / 32.0).astype(
        np.float32
    )
    d = (rng.standard_normal((R, D_OUT), dtype=np.float32) * 0.01).astype(np.float32)
    out = kernel(x=x, weight=w, disha=d)
    print(out.shape, out.dtype)

